# revision 1
# baseline (speedup 1.0000x reference)
"""DeepseekV2 MLA attention (weight-absorbed, MQA-style latent) on 8 TRN2 NeuronCores.

Sharding: data-parallel over batch (B=2) x tensor-parallel over heads (4 heads/core).
Each core computes, for its batch element and its 4 heads, the partial o_proj
output out_t = [HID, S] (transposed layout); the host sums the 4 partials per
batch element and transposes back.

Device kernel layout conventions (per core):
  hidden^T [HID, S] streamed from DRAM.  All projections produce "transposed"
  activations with the output feature on partitions:
    ckv^T [c=512, S], k_pe^T [64, S], q_nope^T [128, S], q_pe^T packs [128, S].
  RoPE is evaluated as q_rot = q_a * cos + q_b * sin where q_a / q_b are two
  projections whose weights were pre-permuted (interleave + rotate-half with
  sign folded) on the host, so no cross-partition ops are needed.
  Scores are computed transposed: scores^T[t, s] = ckv^T.T-contract - softmax
  runs max-free (score magnitudes are ~5 std; verified on host), with the row
  sum obtained by a ones-vector matmul, so no PE transposes of attention
  weights are needed.
"""
import sys

for _p in ("/opt/trn_rl_repo", "/root/.axon_site/_ro/trn_rl_repo"):
    if _p not in sys.path:
        sys.path.insert(0, _p)

import numpy as np

B, S, HID = 2, 2048, 2048
H, DN, DR, KVR, DV = 16, 128, 64, 512, 128
THETA, EPS = 10000.0, 1e-6
SCALE = float((DN + DR) ** -0.5)
NCORES, HL = 8, 4  # 2 (batch) x 4 (head groups of 4)
CH = 512           # s-chunk width (= max fp32 moving operand)


def build_nc(s=S, hid=HID, reps=1):
    import concourse.bacc as bacc
    import concourse.mybir as mybir
    from concourse import tile

    f32 = mybir.dt.float32
    f32r = mybir.dt.float32r
    Exp = mybir.ActivationFunctionType.Exp
    Sqrt = mybir.ActivationFunctionType.Sqrt
    mult = mybir.AluOpType.mult

    def r(ap):
        return ap.bitcast(f32r)

    NCH = s // CH      # s-chunks
    KT = hid // 128    # contraction tiles over HID
    NT = s // 128      # t-tiles

    nc = bacc.Bacc("TRN2", target_bir_lowering=False, debug=False,
                   enable_asserts=False, num_devices=NCORES)

    hid_d = nc.dram_tensor("hid_t", [hid, s], f32r, kind="ExternalInput").ap()
    wqa_d = nc.dram_tensor("wqall_t", [hid, 768], f32r, kind="ExternalInput").ap()
    wkv_d = nc.dram_tensor("wkv_t", [hid, KVR + 2 * DR], f32r, kind="ExternalInput").ap()
    ln_d = nc.dram_tensor("ln_t", [128, 4], f32, kind="ExternalInput").ap()
    kb_d = nc.dram_tensor("kb", [HL, DN, KVR], f32r, kind="ExternalInput").ap()
    vb_d = nc.dram_tensor("vb_t", [HL, KVR, DV], f32r, kind="ExternalInput").ap()
    wo_d = nc.dram_tensor("wo_t", [HL * DV, hid], f32r, kind="ExternalInput").ap()
    cos_d = nc.dram_tensor("cos_p", [128, s], f32, kind="ExternalInput").ap()
    sin_d = nc.dram_tensor("sin_p", [128, s], f32, kind="ExternalInput").ap()
    mask_d = nc.dram_tensor("masks", [128, 4, CH], f32, kind="ExternalInput").ap()
    ident_d = nc.dram_tensor("ident", [128, 128], f32r, kind="ExternalInput").ap()
    onec_d = nc.dram_tensor("ones_c", [128, 1], f32r, kind="ExternalInput").ap()
    oner_d = nc.dram_tensor("ones_r", [1, 128], f32r, kind="ExternalInput").ap()
    out_d = nc.dram_tensor("out_t", [hid, s], f32, kind="ExternalOutput").ap()

    with tile.TileContext(nc) as tc, \
         nc.allow_low_precision(reason="f32r-typed tiles feed fp32r matmuls; psum accum stays fp32"):
        with tc.tile_pool(name="res", bufs=1) as res, \
             tc.tile_pool(name="psp", bufs=8, space="PSUM") as psp:

            def ps_tile(name):
                return psp.tile([128, CH], f32, tag="ps", name=name)

            # resident tiles
            ckvT = [res.tile([128, s], f32r, name=f"ckvT{ci}") for ci in range(4)]
            kper = res.tile([128, s], f32r, name="kper")
            ckvN = [res.tile([128, KVR], f32r, name=f"ckvN{t}") for t in range(NT)]
            kb_sb = res.tile([128, HL, KVR], f32r, name="kb_sb")
            vb_sb = res.tile([128, HL, 4, DV], f32r, name="vb_sb")
            masks = res.tile([128, 4, CH], f32, name="masks_sb")
            ident = res.tile([128, 128], f32r, name="ident_sb")
            onec = res.tile([128, 1], f32r, name="onec_sb")
            oner = res.tile([1, 128], f32r, name="oner_sb")
            ln_sb = res.tile([128, 4], f32, name="ln_sb")
            zb128 = res.tile([128, 1], f32, name="zb128")
            epsb = res.tile([1, 1], f32, name="epsb")
            nc.vector.memset(zb128[:], 0.0)
            nc.vector.memset(epsb[:], EPS)

            nc.scalar.dma_start(ident[:], ident_d)
            nc.scalar.dma_start(onec[:], onec_d)
            nc.scalar.dma_start(oner[:], oner_d)
            nc.scalar.dma_start(ln_sb[:], ln_d)

            # ---------------- pass 1: latent KV (ckv^T, ckv_nat, k_pe rot) ----
            for _rep in range(reps):
              with tc.tile_pool(name="p1", bufs=1) as p1:
                wkv_sb = p1.tile([128, KT, KVR + 2 * DR], f32r, name="wkv_sb")

                prev_post = None
                for j in range(NCH):
                    sl = slice(j * CH, (j + 1) * CH)
                    cos1 = p1.tile([128, CH], f32, tag="cos1", bufs=2, name="cos1")
                    sin1 = p1.tile([128, CH], f32, tag="sin1", bufs=2, name="sin1")
                    nc.scalar.dma_start(cos1[:], cos_d[:, sl])
                    nc.scalar.dma_start(sin1[:], sin_d[:, sl])

                    cps = [ps_tile(f"cps{ci}") for ci in range(4)]
                    kp_ps = ps_tile("kp_ps")
                    for kg in range(KT // 2):
                        ht1 = p1.tile([128, 2, CH], f32r, tag="ht1", bufs=2, name="ht1")
                        nc.sync.dma_start(ht1[:], hid_d[kg * 256:(kg + 1) * 256, sl]
                                          .rearrange("(g p) t -> p g t", p=128))
                        if j == 0:
                            for k0 in (2 * kg, 2 * kg + 1):
                                nc.sync.dma_start(wkv_sb[:, k0, :],
                                                  wkv_d[k0 * 128:(k0 + 1) * 128, :])
                        for ki in range(2):
                            k = 2 * kg + ki
                            st_, sp_ = (k == 0), (k == KT - 1)
                            for ci in range(4):
                                nc.tensor.matmul(cps[ci][:], r(wkv_sb[:, k, ci * 128:(ci + 1) * 128]),
                                                 r(ht1[:, ki, :]), start=st_, stop=sp_)
                            nc.tensor.matmul(kp_ps[:], r(wkv_sb[:, k, KVR:KVR + 128]),
                                             r(ht1[:, ki, :]), start=st_, stop=sp_)

                    # evacuate raw ckv^T to SBUF promptly (frees the 4 cps banks so
                    # the next chunk's k-loop can start); defer the rest of this
                    # chunk's post-processing behind that k-loop.
                    c_sb = []
                    for ci in range(4):
                        t = p1.tile([128, CH], f32, tag="c_sb", bufs=8, name=f"c_sb{ci}")
                        nc.scalar.copy(t[:], cps[ci][:])
                        c_sb.append(t)

                    def make_post(j, sl, c_sb, kp_ps, cos1, sin1):
                        def post():
                            # RMSNorm over c (partition direction) via ones-matmul
                            var_ps = ps_tile("var_ps")
                            for ci in range(4):
                                sqt = p1.tile([128, CH], f32r, tag="sqt", bufs=2, name="sqt")
                                nc.vector.tensor_mul(sqt[:], c_sb[ci][:], c_sb[ci][:])
                                nc.tensor.matmul(var_ps[0:1, :], r(onec[:]), r(sqt[:]),
                                                 start=(ci == 0), stop=(ci == 3))
                            sd1 = p1.tile([1, CH], f32, tag="sd1", bufs=2, name="sd1")
                            nc.scalar.activation(sd1[:], var_ps[0:1, :], Sqrt, bias=epsb[:],
                                                 scale=1.0 / KVR)
                            iv1 = p1.tile([1, CH], f32r, tag="iv1", bufs=2, name="iv1")
                            nc.vector.reciprocal(iv1[:], sd1[:])
                            bc_ps = ps_tile("bc_ps")
                            nc.tensor.matmul(bc_ps[:], r(oner[:]), r(iv1[:]), start=True, stop=True)
                            for ci in range(4):
                                nc.vector.scalar_tensor_tensor(ckvT[ci][:, sl], c_sb[ci][:],
                                                               ln_sb[:, ci:ci + 1], bc_ps[:],
                                                               op0=mult, op1=mult)
                            # k_pe rope: kper = ka*cos + kb*sin (rows 0:64), then duplicate
                            kr_t = p1.tile([128, CH], f32, tag="kr_t", bufs=2, name="kr_t")
                            nc.vector.tensor_mul(kper[0:64, sl], kp_ps[0:64, :], cos1[0:64, :])
                            nc.vector.tensor_mul(kr_t[0:64, :], kp_ps[64:128, :], sin1[0:64, :])
                            nc.vector.tensor_add(kper[0:64, sl], kper[0:64, sl], kr_t[0:64, :])
                            nc.sync.dma_start(kper[64:128, sl], kper[0:64, sl])

                            # transpose normed ckv^T -> ckv natural [t, c]
                            for ss in range(4):
                                t_i = 4 * j + ss
                                for ci in range(4):
                                    tp_ps = ps_tile("tp_ps")
                                    nc.tensor.transpose(r(tp_ps[:, 0:128]),
                                                        ckvT[ci][:, t_i * 128:(t_i + 1) * 128],
                                                        ident[:])
                                    nc.scalar.copy(ckvN[t_i][:, ci * 128:(ci + 1) * 128],
                                                   tp_ps[:, 0:128])
                        return post

                    post_j = make_post(j, sl, c_sb, kp_ps, cos1, sin1)
                    if prev_post is not None:
                        prev_post()
                    prev_post = post_j
                prev_post()

              nc.scalar.dma_start(kb_sb[:], kb_d.rearrange("h d c -> d h c"))
              nc.scalar.dma_start(vb_sb[:], vb_d.rearrange("h (ci p) d -> p h ci d", p=128))
              nc.scalar.dma_start(masks[:], mask_d)

              # ---------------- pass 2: q proj + attention + o_proj -----------
              with tc.tile_pool(name="p2", bufs=1) as p2:
                for j in range(NCH):
                    sl = slice(j * CH, (j + 1) * CH)
                    cos2 = p2.tile([128, CH], f32, tag="cos2", bufs=1, name="cos2")
                    sin2 = p2.tile([128, CH], f32, tag="sin2", bufs=1, name="sin2")
                    nc.scalar.dma_start(cos2[:], cos_d[:, sl])
                    nc.scalar.dma_start(sin2[:], sin_d[:, sl])

                    qn_ps = [ps_tile(f"qn_ps{h}") for h in range(HL)]
                    qa_ps = [ps_tile(f"qa_ps{p}") for p in range(2)]
                    for kg in range(KT // 2):
                        ht2 = p2.tile([128, 2, CH], f32r, tag="ht2", bufs=3, name="ht2")
                        nc.sync.dma_start(ht2[:], hid_d[kg * 256:(kg + 1) * 256, sl]
                                          .rearrange("(g p) t -> p g t", p=128))
                        wq_sb = p2.tile([128, 2, 768], f32r, tag="wq_sb", bufs=3, name="wq_sb")
                        nc.sync.dma_start(wq_sb[:], wqa_d[kg * 256:(kg + 1) * 256, :]
                                          .rearrange("(g p) n -> p g n", p=128))
                        for ki in range(2):
                            k = 2 * kg + ki
                            st_, sp_ = (k == 0), (k == KT - 1)
                            for h in range(HL):
                                nc.tensor.matmul(qn_ps[h][:], r(wq_sb[:, ki, h * 128:(h + 1) * 128]),
                                                 r(ht2[:, ki, :]), start=st_, stop=sp_)
                            for p in range(2):
                                nc.tensor.matmul(qa_ps[p][:], r(wq_sb[:, ki, 512 + p * 128:512 + (p + 1) * 128]),
                                                 r(ht2[:, ki, :]), start=st_, stop=sp_)

                    # evacuate q_nope, rope q_pe
                    qn_sb = []
                    for h in range(HL):
                        t = p2.tile([128, CH], f32r, tag="qn_sb", bufs=4, name=f"qn_sb{h}")
                        nc.scalar.copy(t[:], qn_ps[h][:])
                        qn_sb.append(t)
                    qpr = []
                    for p in range(2):
                        # rotate-half of qa via sign-flipped cross-partition copies
                        qb_sb = p2.tile([128, CH], f32, tag="qb_sb", bufs=2, name="qb_sb")
                        for base in (0, 64):
                            nc.vector.tensor_scalar_mul(qb_sb[base:base + 32, :],
                                                        qa_ps[p][base + 32:base + 64, :], -1.0)
                            nc.vector.tensor_scalar_mul(qb_sb[base + 32:base + 64, :],
                                                        qa_ps[p][base:base + 32, :], 1.0)
                        t = p2.tile([128, CH], f32r, tag="qpr", bufs=2, name=f"qpr{p}")
                        qr_t = p2.tile([128, CH], f32, tag="qr_t", bufs=1, name="qr_t")
                        nc.vector.tensor_mul(t[:], qa_ps[p][:], cos2[:])
                        nc.vector.tensor_mul(qr_t[:], qb_sb[:], sin2[:])
                        nc.vector.tensor_add(t[:], t[:], qr_t[:])
                        qpr.append(t)

                    vo_sb = p2.tile([128, HL, CH], f32r, tag="vo_sb", bufs=1, name="vo_sb")
                    prev_tail = None
                    for h in range(HL):
                        # q_lat^T[c, s] per head
                        ql_sb = p2.tile([128, 4, CH], f32r, tag="ql_sb", bufs=2, name="ql_sb")
                        for ci in range(4):
                            ql_ps = ps_tile("ql_ps")
                            nc.tensor.matmul(ql_ps[:], r(kb_sb[:, h, ci * 128:(ci + 1) * 128]),
                                             r(qn_sb[h][:]), start=True, stop=True)
                            nc.scalar.copy(ql_sb[:, ci, :], ql_ps[:])

                        hp, hh = h // 2, (h % 2) * 64
                        ol_ps = []
                        rs_box = []
                        # t-tile order: diagonal tiles first (first is full-width,
                        # carries start=True), then the off-diagonal history tiles.
                        tts = list(range(4 * j, 4 * j + 4)) + list(range(0, 4 * j))

                        def score_exp(idx):
                            t_i = tts[idx]
                            kd = t_i - 4 * j
                            st = 0 if kd < 0 else (0, 128, 256, 256)[kd]
                            sc_ps = ps_tile("sc_ps")
                            for ci in range(4):
                                nc.tensor.matmul(sc_ps[:, st:], r(ckvT[ci][:, t_i * 128:(t_i + 1) * 128]),
                                                 r(ql_sb[:, ci, st:]), start=(ci == 0), stop=False)
                            nc.tensor.matmul(sc_ps[:, st:],
                                             r(kper[hh:hh + 64, t_i * 128:(t_i + 1) * 128]),
                                             r(qpr[hp][hh:hh + 64, st:]), start=False, stop=True)
                            if kd >= 0:
                                nc.vector.tensor_add(sc_ps[:, st:], sc_ps[:, st:], masks[:, kd, st:])
                            ex_sb = p2.tile([128, CH], f32r, tag="ex_sb", bufs=4, name="ex_sb")
                            nc.scalar.activation(ex_sb[:, st:], sc_ps[:, st:], Exp,
                                                 bias=zb128[:], scale=SCALE)
                            return ex_sb, st

                        def pv(idx, ex_sb, st):
                            t_i = tts[idx]
                            first, last = (idx == 0), (idx == len(tts) - 1)
                            for ci in range(4):
                                nc.tensor.matmul(ol_ps[ci][:, st:], r(ckvN[t_i][:, ci * 128:(ci + 1) * 128]),
                                                 r(ex_sb[:, st:]), start=first, stop=last)
                            nc.tensor.matmul(rs_box[0][0:1, st:], r(onec[:]), r(ex_sb[:, st:]),
                                             start=first, stop=last)

                        # overlap previous head's tail behind this head's q_lat and
                        # first two score tiles (ol/rs banks allocate only after the
                        # previous head's are released inside prev_tail)
                        npre = min(3, len(tts))
                        pends = [(i,) + score_exp(i) for i in range(npre)]
                        if prev_tail is not None:
                            prev_tail()
                            prev_tail = None
                        ol_ps.extend(ps_tile(f"ol_ps{ci}") for ci in range(4))
                        rs_box.append(ps_tile("rs_ps"))
                        for idx in range(npre, len(tts)):
                            pends.append((idx,) + score_exp(idx))
                            if len(pends) > 5:
                                pv(*pends.pop(0))
                        for pend in pends:
                            pv(*pend)

                        def make_tail(h, ol_ps, rs_ps):
                            def tail():
                                # evacuate unnormalized out_lat (starts right after last PV)
                                ol_sb = p2.tile([128, 4, CH], f32r, tag="ol_sb", bufs=1, name="ol_sb")
                                for ci in range(4):
                                    nc.scalar.copy(ol_sb[:, ci, :], ol_ps[ci][:])
                                # softmax denominator -> broadcast tile (parallel chain)
                                rv_sb = p2.tile([1, CH], f32r, tag="rv_sb", bufs=1, name="rv_sb")
                                nc.vector.reciprocal(rv_sb[:], rs_ps[0:1, :])
                                bc2_ps = ps_tile("bc2_ps")
                                nc.tensor.matmul(bc2_ps[:], r(oner[:]), r(rv_sb[:]), start=True, stop=True)
                                bc2_sb = p2.tile([128, CH], f32, tag="bc2_sb", bufs=1, name="bc2_sb")
                                nc.scalar.copy(bc2_sb[:], bc2_ps[:])
                                # v_b expansion on unnormalized out_lat; normalize once on
                                # v_out (per-column scaling commutes with the contraction)
                                vo_ps = ps_tile("vo_ps")
                                for ci in range(4):
                                    nc.tensor.matmul(vo_ps[:], r(vb_sb[:, h, ci, :]), r(ol_sb[:, ci, :]),
                                                     start=(ci == 0), stop=(ci == 3))
                                nc.vector.tensor_mul(vo_sb[:, h, :], vo_ps[:], bc2_sb[:])
                            return tail

                        prev_tail = make_tail(h, ol_ps, rs_box[0])
                    prev_tail()

                    # o_proj partial: out^T[hid, s] = sum_h wo^T.T @ v_out^T
                    for htile in range(KT):
                        wo_sb = p2.tile([128, HL, 128], f32r, tag="wo_sb", bufs=3, name="wo_sb")
                        nc.sync.dma_start(wo_sb[:], wo_d[:, htile * 128:(htile + 1) * 128]
                                          .rearrange("(a p) n -> p a n", p=128))
                        oo_ps = ps_tile("oo_ps")
                        for hh2 in range(HL):
                            nc.tensor.matmul(oo_ps[:], r(wo_sb[:, hh2, :]), r(vo_sb[:, hh2, :]),
                                             start=(hh2 == 0), stop=(hh2 == HL - 1))
                        oo_sb = p2.tile([128, CH], f32, tag="oo_sb", bufs=3, name="oo_sb")
                        nc.vector.tensor_copy(oo_sb[:], oo_ps[:])
                        nc.scalar.dma_start(out_d[htile * 128:(htile + 1) * 128, sl], oo_sb[:])

    nc.compile()
    return nc


# ---------------------------------------------------------------------------
# host-side input prep / output assembly
# ---------------------------------------------------------------------------
_PERM = np.concatenate([np.arange(0, DR, 2), np.arange(1, DR, 2)])


def _rope_tables(pos, s):
    inv_freq = 1.0 / (THETA ** (np.arange(0, DR, 2, dtype=np.float64) / DR))
    t = pos.astype(np.float64)
    freqs = t[:, None] * inv_freq
    emb = np.concatenate([freqs, freqs], axis=-1)          # [s, DR]
    cosT = np.cos(emb).T.astype(np.float32)                # [DR, s]
    sinT = np.sin(emb).T.astype(np.float32)
    cos_p = np.ascontiguousarray(np.vstack([cosT, cosT]))  # [128, s]
    sin_p = np.ascontiguousarray(np.vstack([sinT, sinT]))
    return cos_p, sin_p


def _masks():
    t = np.arange(128)[:, None]
    c = np.arange(CH)[None, :]
    m = np.zeros((128, 4, CH), np.float32)
    for kd in range(4):
        m[:, kd, :] = np.where(c >= 128 * kd + t, 0.0, -1e30).astype(np.float32)
    return m


def prep_core_inputs(inputs, core, s=S, hid=HID):
    b, g = core // 4, core % 4
    heads = slice(HL * g, HL * (g + 1))
    hs = np.asarray(inputs["hidden_states"], np.float32)[b, :s, :hid]
    m = {"hid_t": np.ascontiguousarray(hs.T)}

    wq = np.asarray(inputs["q_nope_weight"], np.float32).reshape(H, DN, HID)[heads, :, :hid]
    wq_t = wq.transpose(2, 0, 1).reshape(hid, HL * DN)

    wqp = np.asarray(inputs["q_pe_weight"], np.float32).reshape(H, DR, HID)[heads, :, :hid]
    a = wqp[:, _PERM, :]                                   # [4, 64, hid]
    bv = np.concatenate([-a[:, 32:64], a[:, 0:32]], axis=1)
    A = a.reshape(2, 128, hid)
    Bv = bv.reshape(2, 128, hid)
    wqpe_t = np.concatenate([A[0], A[1]], axis=0).T
    m["wqall_t"] = np.ascontiguousarray(np.concatenate([wq_t, wqpe_t], axis=1))

    wkv = np.asarray(inputs["kv_a_weight"], np.float32)[:, :hid]
    kpe_a = wkv[KVR:][_PERM]
    kpe_b = np.concatenate([-kpe_a[32:], kpe_a[:32]], axis=0)
    m["wkv_t"] = np.ascontiguousarray(np.concatenate([wkv[:KVR], kpe_a, kpe_b], axis=0).T)

    m["ln_t"] = np.ascontiguousarray(
        np.asarray(inputs["kv_a_ln_weight"], np.float32).reshape(4, 128).T)
    m["kb"] = np.ascontiguousarray(np.asarray(inputs["k_b_weight"], np.float32)[heads])
    m["vb_t"] = np.ascontiguousarray(
        np.asarray(inputs["v_b_weight"], np.float32)[heads].transpose(0, 2, 1))
    m["wo_t"] = np.ascontiguousarray(
        np.asarray(inputs["o_weight"], np.float32)[:hid, HL * DV * g:HL * DV * (g + 1)].T)

    pos = np.asarray(inputs["position_ids"]).reshape(-1)[:s]
    cos_p, sin_p = _rope_tables(pos, s)
    m["cos_p"], m["sin_p"] = cos_p, sin_p
    m["masks"] = _masks()
    m["ident"] = np.eye(128, dtype=np.float32)
    m["ones_c"] = np.ones((128, 1), np.float32)
    m["ones_r"] = np.ones((1, 128), np.float32)
    return m


_NC_CACHE = {}


def _get_nc():
    if "nc" not in _NC_CACHE:
        _NC_CACHE["nc"] = build_nc()
    return _NC_CACHE["nc"]


def kernel(**inputs):
    from concourse import bass_utils

    nc = _get_nc()
    in_maps = [prep_core_inputs(inputs, c) for c in range(NCORES)]
    res = bass_utils.run_bass_kernel_spmd(nc, in_maps, core_ids=list(range(NCORES)))
    out = np.empty((B, S, HID), np.float32)
    for b in range(B):
        acc = np.array(res.results[4 * b]["out_t"], np.float32)
        for g in range(1, 4):
            acc += res.results[4 * b + g]["out_t"]
        out[b] = acc.T
    return out



# revision 49
# speedup vs baseline: 1.3785x; 1.3785x over previous
"""DeepseekV2 MLA attention (weight-absorbed, MQA-style latent) on 8 TRN2 NeuronCores.

Sharding: data-parallel over batch (B=2) x tensor-parallel over heads (4 heads/core).
Each core computes, for its batch element and its 4 heads, the partial o_proj
output out_t = [HID, S] (transposed layout); the host sums the 4 partials per
batch element and transposes back.

Mixed-precision design.  The only fast PE mode is fp8e4m3 with DoubleRow
(2x128-deep contraction per instruction at 0.5 cycles/row), so:
  * Q-path (q projections, q_lat, rope, score matmuls) runs in plain fp8 —
    its quantization noise enters through the softmax exponent and is
    strongly damped (measured contribution ~4e-3).
  * V-path values must be accurate: the ckv projection and o_proj use fp8 +
    fp8-residual splits (x = x8 + r8), i.e. 3-term DoubleRow products
    (x8*y8 + x8*ry + rx*y8) keeping ~0.1-0.5% error at 1.5x the plain-fp8
    cost.  The PV matmul and v_b expansion run in fp16 (1.0 cycles/row) so
    the exp() output feeds the PV matmul directly with no per-tile
    requantization chain.

Scale ledger (log2 of stored/true): hid8 0 | wq/wkv/kb/vb/wo +5 | qn8 +2 |
ql8 +5 | qpr +5 | kper 0 (k-rope tables pre-divided by 32) | ckv latents 0
(RMSNorm cancels +5; eps pre-scaled 2^10) | scores +5 (exp scale SCALE/32) |
ex 0 | ol16 +4 normalized | vo16 0 | oo_ps +5 -> out x 2^-5.

Softmax is max-free (score magnitudes are small; verified on host); the
denominator comes from DoubleRow ones-matmuls over the fp8 ex + residual.
"""
import sys

for _p in ("/opt/trn_rl_repo", "/root/.axon_site/_ro/trn_rl_repo"):
    if _p not in sys.path:
        sys.path.insert(0, _p)

import numpy as np
import ml_dtypes

B, S, HID = 2, 2048, 2048
H, DN, DR, KVR, DV = 16, 128, 64, 512, 128
THETA, EPS = 10000.0, 1e-6
SCALE = float((DN + DR) ** -0.5)
NCORES, HL = 8, 4  # 2 (batch) x 4 (head groups of 4)
CH = 512           # s-chunk width (= psum bank width in fp32)
WS = 32.0          # host-side weight pre-scale (2^5)

FP8 = ml_dtypes.float8_e4m3
BF16 = ml_dtypes.bfloat16


def build_nc(s=S, hid=HID, reps=1):
    import concourse.bacc as bacc
    import concourse.mybir as mybir
    from concourse import tile

    f32 = mybir.dt.float32
    f32r = mybir.dt.float32r
    fp8 = mybir.dt.float8e4
    fp16 = mybir.dt.float16
    bf16 = mybir.dt.bfloat16
    Exp = mybir.ActivationFunctionType.Exp
    Sqrt = mybir.ActivationFunctionType.Sqrt
    Copy = mybir.ActivationFunctionType.Copy
    mult = mybir.AluOpType.mult
    subtract = mybir.AluOpType.subtract
    DRow = mybir.MatmulPerfMode.DoubleRow

    def r(ap):
        return ap.bitcast(f32r)

    NCH = s // CH      # s-chunks
    KT = hid // 128    # contraction tiles over HID
    KG = KT // 2       # DoubleRow contraction pairs
    NT = s // 128      # t-tiles
    NPR = NT // 2      # t-tile pairs

    nc = bacc.Bacc("TRN2", target_bir_lowering=False, debug=False,
                   enable_asserts=False, num_devices=NCORES)

    hid_d = nc.dram_tensor("hid8", [hid, s], fp8, kind="ExternalInput").ap()
    hidr_d = nc.dram_tensor("hidr8", [hid, s], fp8, kind="ExternalInput").ap()
    wq_d = nc.dram_tensor("wq8", [hid, 768], fp8, kind="ExternalInput").ap()
    wkv_d = nc.dram_tensor("wkv8", [hid, KVR + 2 * DR], fp8, kind="ExternalInput").ap()
    wkvr_d = nc.dram_tensor("wkvr8", [hid, KVR + 2 * DR], fp8, kind="ExternalInput").ap()
    ln_d = nc.dram_tensor("ln_t", [128, 4], f32, kind="ExternalInput").ap()
    kb_d = nc.dram_tensor("kb16", [HL, DN, KVR], fp16, kind="ExternalInput").ap()
    vb_d = nc.dram_tensor("vb16_t", [HL, KVR, DV], fp16, kind="ExternalInput").ap()
    wo_d = nc.dram_tensor("wo8_t", [HL * DV, hid], fp8, kind="ExternalInput").ap()
    wor_d = nc.dram_tensor("wor8_t", [HL * DV, hid], fp8, kind="ExternalInput").ap()
    cs2_d = nc.dram_tensor("cs2", [128, s], bf16, kind="ExternalInput").ap()
    ckr_d = nc.dram_tensor("ckrope", [128, s], bf16, kind="ExternalInput").ap()
    mask_d = nc.dram_tensor("masks", [128, 4, CH], fp16, kind="ExternalInput").ap()
    id16_d = nc.dram_tensor("ident16", [128, 128], fp16, kind="ExternalInput").ap()
    onec_d = nc.dram_tensor("ones_c", [128, 1], f32r, kind="ExternalInput").ap()
    oner_d = nc.dram_tensor("ones_r", [1, 128], f32r, kind="ExternalInput").ap()
    on16_d = nc.dram_tensor("ones16", [128, 32], fp16, kind="ExternalInput").ap()
    on8_d = nc.dram_tensor("ones8", [128, 2, 32], fp8, kind="ExternalInput").ap()
    out_d = nc.dram_tensor("out_t", [hid, s], f32, kind="ExternalOutput").ap()

    with tile.TileContext(nc) as tc, \
         nc.allow_low_precision(reason="fp8/fp16 matmuls; psum accum stays fp32"):
        with tc.tile_pool(name="res", bufs=1) as res, \
             tc.tile_pool(name="psp", bufs=8, space="PSUM") as psp:

            def ps_tile(name):
                return psp.tile([128, CH], f32, tag="ps", name=name)

            # resident tiles
            hid8 = res.tile([128, KT, s], fp8, name="hid8_sb")
            wq8 = res.tile([128, KT, 768], fp8, name="wq8_sb")
            wkv8 = res.tile([128, KT, KVR + 2 * DR], fp8, name="wkv8_sb")
            wkvr = res.tile([128, KT, KVR + 2 * DR], fp8, name="wkvr_sb")
            wo8 = res.tile([128, HL, hid], fp8, name="wo8_sb")
            wor = res.tile([128, HL, hid], fp8, name="wor_sb")
            ckvT8 = res.tile([128, 4, s], fp8, name="ckvT8")
            ckvN16 = [res.tile([128, 2, KVR], fp16, name=f"ckvN16_{m}") for m in range(NPR)]
            kper16 = res.tile([64, s], fp16, name="kper16")
            kb16 = res.tile([128, HL, KVR], fp16, name="kb16_sb")
            vb16 = res.tile([128, HL, 4, DV], fp16, name="vb16_sb")
            cs2 = res.tile([128, s], bf16, name="cs2_sb")
            ckr = res.tile([128, s], bf16, name="ckr_sb")
            masks = res.tile([128, 4, CH], fp16, name="masks_sb")
            ln_sb = res.tile([128, 4], f32, name="ln_sb")
            id16 = res.tile([128, 128], fp16, name="id16_sb")
            onec = res.tile([128, 1], f32r, name="onec_sb")
            oner = res.tile([1, 128], f32r, name="oner_sb")
            on16 = res.tile([128, 32], fp16, name="on16_sb")
            on8 = res.tile([128, 2, 32], fp8, name="on8_sb")
            zb128 = res.tile([128, 1], f32, name="zb128")
            epsb = res.tile([1, 1], f32, name="epsb")
            c16 = res.tile([128, 1], f32, name="c16")
            c1_16 = res.tile([128, 1], f32, name="c1_16")
            cm1 = res.tile([128, 1], f32, name="cm1")
            nc.vector.memset(zb128[:], 0.0)
            nc.vector.memset(epsb[:], EPS * WS * WS)
            nc.vector.memset(c16[:], 16.0)
            nc.vector.memset(c1_16[:], 0.0625)
            nc.vector.memset(cm1[:], -1.0)

            # DMA queue assignment: sync/scalar carry the pass-1-critical loads
            # (first matmul needs wkv8 + hid8[j0] + hidr[j0]); pass-2-only
            # residents ride the pool queue (Pool engine has slack; SWDGE
            # desc-gen costs ~1us of Pool ENGINE time per copy).
            nc.sync.dma_start(wkv8[:], wkv_d.rearrange("(g p) n -> p g n", p=128))
            for j in range(NCH):
                sl = slice(j * CH, (j + 1) * CH)
                nc.scalar.dma_start(hid8[:, :, sl],
                                    hid_d[:, sl].rearrange("(g p) t -> p g t", p=128))
            nc.scalar.dma_start(ckr[:], ckr_d)
            nc.scalar.dma_start(ln_sb[:], ln_d)
            nc.scalar.dma_start(id16[:], id16_d)
            nc.scalar.dma_start(onec[:], onec_d)
            nc.scalar.dma_start(oner[:], oner_d)
            nc.scalar.dma_start(on16[:], on16_d)
            nc.scalar.dma_start(on8[:], on8_d)
            nc.gpsimd.dma_start(cs2[:], cs2_d)
            nc.gpsimd.dma_start(wq8[:], wq_d.rearrange("(g p) n -> p g n", p=128))
            nc.gpsimd.dma_start(kb16[:], kb_d.rearrange("h d c -> d h c"))
            nc.gpsimd.dma_start(vb16[:], vb_d.rearrange("h (ci p) d -> p h ci d", p=128))
            nc.gpsimd.dma_start(masks[:], mask_d)
            nc.gpsimd.dma_start(wo8[:], wo_d.rearrange("(a p) n -> p a n", p=128))
            nc.gpsimd.dma_start(wor[:], wor_d.rearrange("(a p) n -> p a n", p=128))

            for _rep in range(reps):
              # ---------------- pass 1: latent KV (ckvT8, ckvN8+r, k_pe rot) --
              with tc.tile_pool(name="p1", bufs=1) as p1:
                prev_post = None
                for j in range(NCH):
                    sl = slice(j * CH, (j + 1) * CH)
                    hidr = p1.tile([128, KT, CH], fp8, tag="hidr", bufs=2, name="hidr")
                    nc.sync.dma_start(hidr[:],
                                      hidr_d[:, sl].rearrange("(g p) t -> p g t", p=128))
                    if j == 0:
                        nc.sync.dma_start(wkvr[:],
                                          wkvr_d.rearrange("(g p) n -> p g n", p=128))
                    cps = [ps_tile(f"cps{ci}") for ci in range(4)]
                    kp_ps = ps_tile("kp_ps")
                    nmb = (KVR + 2 * DR) // 128
                    # term-major order staggers the hidr/wkvr first-uses so the
                    # startup DMAs for them overlap the wkv8*hid8 matmuls
                    for term in range(3):
                        for kg in range(KG):
                            st_ = (term == 0 and kg == 0)
                            sp_ = (term == 2 and kg == KG - 1)
                            kk = slice(2 * kg, 2 * kg + 2)
                            w_t, h_t = ((wkv8[:, kk, :], hid8[:, kk, sl]),
                                        (wkv8[:, kk, :], hidr[:, kk, :]),
                                        (wkvr[:, kk, :], hid8[:, kk, sl]))[term]
                            for mb in range(nmb):
                                mbs = slice(mb * 128, (mb + 1) * 128)
                                out = cps[mb][:] if mb < 4 else kp_ps[:]
                                nc.tensor.matmul(out, w_t[:, :, mbs], h_t,
                                                 start=st_, stop=sp_, perf_mode=DRow)

                    # evacuate raw ckv^T + k_pe to SBUF promptly (frees psum for
                    # the next chunk's k-loop); defer the chunk's post-processing.
                    c_sb = []
                    for ci in range(4):
                        t = p1.tile([128, CH], f32r, tag="c_sb", bufs=8, name=f"c_sb{ci}")
                        nc.scalar.copy(t[:], cps[ci][:])
                        c_sb.append(t)
                    kp_sb = p1.tile([128, CH], f32, tag="kp_sb", bufs=2, name="kp_sb")
                    nc.scalar.copy(kp_sb[:], kp_ps[:])

                    def make_post(j, sl, c_sb, kp_sb):
                        def post():
                            # RMSNorm stats over c (partition dir) via ones-matmul
                            var_ps = ps_tile("var_ps")
                            for ci in range(4):
                                sqt = p1.tile([128, CH], f32r, tag="sqt", bufs=2, name="sqt")
                                nc.vector.tensor_mul(sqt[:], c_sb[ci][:], c_sb[ci][:])
                                nc.tensor.matmul(var_ps[0:1, :], r(onec[:]), r(sqt[:]),
                                                 start=(ci == 0), stop=(ci == 3))
                            sd1 = p1.tile([1, CH], f32, tag="sd1", bufs=2, name="sd1")
                            nc.scalar.activation(sd1[:], var_ps[0:1, :], Sqrt, bias=epsb[:],
                                                 scale=1.0 / KVR)
                            iv1 = p1.tile([1, CH], f32r, tag="iv1", bufs=2, name="iv1")
                            nc.vector.reciprocal(iv1[:], sd1[:])
                            bc_ps = ps_tile("bc_ps")
                            nc.tensor.matmul(bc_ps[:], r(oner[:]), r(iv1[:]), start=True, stop=True)
                            ckvT16 = p1.tile([128, 4, CH], fp16, tag="ckvT16", bufs=2,
                                             name="ckvT16")
                            for ci in range(4):
                                nc.vector.scalar_tensor_tensor(ckvT16[:, ci, :], c_sb[ci][:],
                                                               ln_sb[:, ci:ci + 1], bc_ps[:],
                                                               op0=mult, op1=mult)
                                nc.scalar.copy(ckvT8[:, ci, sl], ckvT16[:, ci, :])
                            # k_pe rope: rows 0:64 = a*cos/WS, 64:128 = b*sin/WS.
                            # Products are written to base-0 slabs (partition
                            # shift rides the psum-input ops), then added
                            # partition-aligned.
                            ta_s = p1.tile([32, 2, CH], f32, tag="ta_s", bufs=2, name="ta_s")
                            tb_s = p1.tile([32, 2, CH], f32, tag="tb_s", bufs=2, name="tb_s")
                            for i2 in range(2):
                                nc.vector.tensor_mul(ta_s[:, i2, :],
                                                     kp_sb[32 * i2:32 * i2 + 32, :],
                                                     ckr[32 * i2:32 * i2 + 32, sl])
                                nc.vector.tensor_mul(tb_s[:, i2, :],
                                                     kp_sb[64 + 32 * i2:96 + 32 * i2, :],
                                                     ckr[64 + 32 * i2:96 + 32 * i2, sl])
                                nc.vector.tensor_add(kper16[32 * i2:32 * i2 + 32, sl],
                                                     ta_s[:, i2, :], tb_s[:, i2, :])
                            # natural-layout latent: fp16 transposes -> fp8 + residual
                            for q in range(4):
                                t_i = 4 * j + q
                                m, par = t_i // 2, t_i % 2
                                lb = slice(q * 128, (q + 1) * 128)
                                tp_ps = ps_tile("tp_ps")
                                tp16 = tp_ps.bitcast(fp16)
                                for ci in range(4):
                                    nc.tensor.transpose(tp16[:, ci * 128:(ci + 1) * 128],
                                                        ckvT16[:, ci, lb], id16[:])
                                nc.vector.tensor_copy(ckvN16[m][:, par, :], tp16[:, 0:KVR])
                        return post

                    post_j = make_post(j, sl, c_sb, kp_sb)
                    if prev_post is not None:
                        prev_post()
                    prev_post = post_j
                prev_post()

              # ---------------- pass 2: q proj + attention + o_proj -----------
              with tc.tile_pool(name="p2", bufs=1) as p2:
                prev_oproj = None
                for j in range(NCH):
                    sl = slice(j * CH, (j + 1) * CH)

                    hidr2 = p2.tile([128, KT, CH], fp8, tag="hidr2", bufs=1, name="hidr2")
                    nc.sync.dma_start(hidr2[:],
                                      hidr_d[:, sl].rearrange("(g p) t -> p g t", p=128))
                    qn_ps = [ps_tile(f"qn_ps{h}") for h in range(HL)]
                    qa_ps = [ps_tile(f"qa_ps{p}") for p in range(2)]
                    for term in range(2):
                        h_t = hid8[:, :, sl] if term == 0 else hidr2[:]
                        for kg in range(KG):
                            st_ = (term == 0 and kg == 0)
                            sp_ = (term == 1 and kg == KG - 1)
                            kk = slice(2 * kg, 2 * kg + 2)
                            for h in range(HL):
                                nc.tensor.matmul(qn_ps[h][:], wq8[:, kk, h * 128:(h + 1) * 128],
                                                 h_t[:, kk, :], start=st_, stop=sp_, perf_mode=DRow)
                            for p in range(2):
                                nc.tensor.matmul(qa_ps[p][:], wq8[:, kk, 512 + p * 128:512 + (p + 1) * 128],
                                                 h_t[:, kk, :], start=st_, stop=sp_, perf_mode=DRow)

                    # evacuate q_nope (fp8, x2^-3); rope q_pe into ql8a planes 4/5
                    ql8a = p2.tile([128, 4, HL, CH], fp8, tag="ql8a", bufs=2, name="ql8a")
                    qpr16 = p2.tile([64, HL, CH], fp16, tag="qpr16", bufs=1, name="qpr16")
                    qn16 = []
                    for h in range(HL):
                        t = p2.tile([128, CH], fp16, tag="qn16", bufs=5, name=f"qn16_{h}")
                        nc.scalar.activation(t[:], qn_ps[h][:], Copy, scale=0.125)
                        qn16.append(t)
                    for p in range(2):
                        # q rope: qc = qa*cos; qr = rotate_half(qa)*sin with the
                        # sign flip folded into an stt (cross-partition reads)
                        qc = p2.tile([128, CH], f32, tag="qc", bufs=1, name="qc")
                        qr = p2.tile([128, CH], f32, tag="qr", bufs=1, name="qr")
                        for hh in (0, 64):
                            nc.vector.tensor_mul(qc[hh:hh + 64, :], qa_ps[p][hh:hh + 64, :],
                                                 cs2[0:64, sl])
                            nc.vector.scalar_tensor_tensor(qr[hh:hh + 32, :],
                                                           qa_ps[p][hh + 32:hh + 64, :],
                                                           cm1[64:96, :], cs2[64:96, sl],
                                                           op0=mult, op1=mult)
                            nc.vector.tensor_mul(qr[hh + 32:hh + 64, :],
                                                 qa_ps[p][hh:hh + 32, :], cs2[96:128, sl])
                        for i, hh in ((0, 0), (1, 64)):
                            h2 = 2 * p + i
                            nc.vector.tensor_add(qpr16[0:32, h2, :],
                                                 qc[hh:hh + 32, :], qr[hh:hh + 32, :])
                            nc.vector.tensor_add(qpr16[32:64, h2, :],
                                                 qc[hh + 32:hh + 64, :], qr[hh + 32:hh + 64, :])

                    # previous chunk's o_proj drains here, overlapping this
                    # chunk's q-projection work
                    if prev_oproj is not None:
                        prev_oproj()
                        prev_oproj = None

                    vo8a = p2.tile([128, HL, CH], fp8, tag="vo8a", bufs=2, name="vo8a")
                    vor8 = p2.tile([128, HL, CH], fp8, tag="vor8", bufs=2, name="vor8")
                    prev_tail = None
                    for h in range(HL):
                        # q_lat^T[c, s]: plain fp8 matmuls (K=128), evac x 2^-2
                        for ci in range(4):
                            ql_ps = ps_tile("ql_ps")
                            nc.tensor.matmul(ql_ps[:], kb16[:, h, ci * 128:(ci + 1) * 128],
                                             qn16[h][:], start=True, stop=True)
                            if ci % 2 == 0:
                                nc.scalar.activation(ql8a[:, ci, h, :], ql_ps[:], Copy,
                                                     scale=0.25)
                            else:
                                nc.vector.tensor_scalar_mul(ql8a[:, ci, h, :], ql_ps[:], 0.25)

                        # emit the previous head's tail now so its psum-freeing
                        # chain overlaps this head's ql/score matmuls
                        if prev_tail is not None:
                            prev_tail()
                            prev_tail = None

                        # t-pair order: diagonal pairs first, then history pairs
                        prs = [(2 * j, 0, True), (2 * j + 1, 256, True)] + \
                              [(m, 0, False) for m in range(0, 2 * j)]

                        def do_pair(m, st, diag):
                            e8p = None if diag else p2.tile([128, 2, CH], fp8,
                                                            tag="e8p", bufs=4, name="e8p")
                            exs = []
                            for par in range(2):
                                t_i = 2 * m + par
                                tb = slice(t_i * 128, (t_i + 1) * 128)
                                sc_ps = ps_tile("sc_ps")
                                nc.tensor.matmul(sc_ps[:, st:], ckvT8[:, 0:2, tb],
                                                 ql8a[:, 0:2, h, st:],
                                                 start=True, stop=False, perf_mode=DRow)
                                nc.tensor.matmul(sc_ps[:, st:], ckvT8[:, 2:4, tb],
                                                 ql8a[:, 2:4, h, st:],
                                                 start=False, stop=False, perf_mode=DRow)
                                if diag:
                                    # mask add as a tiny fp16 identity-matmul on
                                    # the PE, folded into the score accumulation
                                    kd = t_i - 4 * j
                                    ma, mb2 = ((0, 128), (0, 256),
                                               (256, 384), (256, 512))[kd]
                                    nc.tensor.matmul(sc_ps[:, ma:mb2], id16[:],
                                                     masks[:, kd, ma:mb2],
                                                     start=False, stop=False)
                                nc.tensor.matmul(sc_ps[:, st:], kper16[:, tb],
                                                 qpr16[:, h, st:],
                                                 start=False, stop=True)
                                ex16 = p2.tile([128, CH], fp16, tag="ex16", bufs=6, name="ex16")
                                nc.scalar.activation(ex16[:, st:], sc_ps[:, st:], Exp,
                                                     bias=zb128[:], scale=SCALE / WS)
                                if e8p is not None:
                                    nc.gpsimd.tensor_copy(e8p[:, par, :], ex16[:])
                                exs.append(ex16)
                            return tuple(exs) + (e8p,)

                        ol_ps = []
                        rs_box = []

                        def pv(idx, m, st, ex_a, ex_b, e8p):
                            first, last = (idx == 0), (idx == len(prs) - 1)
                            for par, ext in ((0, ex_a), (1, ex_b)):
                                for ci in range(4):
                                    cb = slice(ci * 128, (ci + 1) * 128)
                                    nc.tensor.matmul(ol_ps[ci][:, st:],
                                                     ckvN16[m][:, par, cb], ext[:, st:],
                                                     start=(first and par == 0),
                                                     stop=(last and par == 1))
                                if e8p is None:
                                    nc.tensor.matmul(rs_box[0][0:32, st:], on16[:], ext[:, st:],
                                                     start=(first and par == 0),
                                                     stop=(last and par == 1))
                            if e8p is not None:
                                nc.tensor.matmul(rs_box[0][0:32, :], on8[:], e8p[:],
                                                 start=first, stop=last, perf_mode=DRow)

                        pend = []
                        for idx, (m, st, diag) in enumerate(prs):
                            pair_t = do_pair(m, st, diag)
                            if idx == 0:
                                ol_ps.extend(ps_tile(f"ol_ps{ci}") for ci in range(4))
                                rs_box.append(ps_tile("rs_ps"))
                            pend.append((idx, m, st) + pair_t)
                            if len(pend) > 3:
                                pv(*pend.pop(0))
                        for pd in pend:
                            pv(*pd)

                        def make_tail(h, ol_ps, rs_ps):
                            def tail():
                                # softmax denominator -> broadcast tile
                                rv = p2.tile([1, CH], f32r, tag="rv", bufs=1, name="rv")
                                nc.vector.reciprocal(rv[:], rs_ps[0:1, :])
                                bc2_ps = ps_tile("bc2_ps")
                                nc.tensor.matmul(bc2_ps[:], r(oner[:]), r(rv[:]), start=True, stop=True)
                                bc2_sb = p2.tile([128, CH], f32, tag="bc2_sb", bufs=2, name="bc2_sb")
                                nc.scalar.copy(bc2_sb[:], bc2_ps[:])
                                # normalized out_lat -> fp16 (x16), v_b expansion fp16
                                ol16 = p2.tile([128, 4, CH], fp16, tag="ol16", bufs=2, name="ol16")
                                for ci in range(4):
                                    nc.vector.scalar_tensor_tensor(ol16[:, ci, :], ol_ps[ci][:],
                                                                   c16[:], bc2_sb[:],
                                                                   op0=mult, op1=mult)
                                vo_ps = ps_tile("vo_ps")
                                for ci in range(4):
                                    nc.tensor.matmul(vo_ps[:], vb16[:, h, ci, :], ol16[:, ci, :],
                                                     start=(ci == 0), stop=(ci == 3))
                                nc.scalar.activation(vo8a[:, h, :], vo_ps[:], Copy, scale=0.0625)
                                nc.vector.scalar_tensor_tensor(vor8[:, h, :], vo_ps[:],
                                                               c1_16[:], vo8a[:, h, :],
                                                               op0=mult, op1=subtract)
                            return tail

                        prev_tail = make_tail(h, ol_ps, rs_box[0])
                    prev_tail()

                    # o_proj partial (3-term fp8x2): out^T = sum_h wo^T.T @ v_out^T
                    def make_oproj(sl, vo8a, vor8):
                        def oproj():
                            for ht in range(KT):
                                htb = slice(ht * 128, (ht + 1) * 128)
                                oo_ps = ps_tile("oo_ps")
                                for g2 in range(2):
                                    hh2 = slice(2 * g2, 2 * g2 + 2)
                                    nc.tensor.matmul(oo_ps[:], wo8[:, hh2, htb], vo8a[:, hh2, :],
                                                     start=(g2 == 0), stop=False, perf_mode=DRow)
                                    nc.tensor.matmul(oo_ps[:], wo8[:, hh2, htb], vor8[:, hh2, :],
                                                     start=False, stop=False, perf_mode=DRow)
                                    nc.tensor.matmul(oo_ps[:], wor[:, hh2, htb], vo8a[:, hh2, :],
                                                     start=False, stop=(g2 == 1), perf_mode=DRow)
                                oo_sb = p2.tile([128, CH], f32, tag="oo_sb", bufs=4, name="oo_sb")
                                if ht % 2 == 0:
                                    nc.scalar.activation(oo_sb[:], oo_ps[:], Copy,
                                                         scale=1.0 / (WS * WS))
                                else:
                                    nc.vector.tensor_scalar_mul(oo_sb[:], oo_ps[:],
                                                                1.0 / (WS * WS))
                                nc.sync.dma_start(out_d[htb, sl], oo_sb[:])
                        return oproj

                    prev_oproj = make_oproj(sl, vo8a, vor8)
                prev_oproj()

    nc.compile()
    return nc


# ---------------------------------------------------------------------------
# host-side input prep / output assembly
# ---------------------------------------------------------------------------
_PERM = np.concatenate([np.arange(0, DR, 2), np.arange(1, DR, 2)])


def _rope_tables(pos, s):
    inv_freq = 1.0 / (THETA ** (np.arange(0, DR, 2, dtype=np.float64) / DR))
    t = pos.astype(np.float64)
    freqs = t[:, None] * inv_freq
    emb = np.concatenate([freqs, freqs], axis=-1)          # [s, DR]
    cosT = np.cos(emb).T.astype(np.float32)                # [DR, s]
    sinT = np.sin(emb).T.astype(np.float32)
    return cosT, sinT


def _masks():
    t = np.arange(128)[:, None]
    c = np.arange(CH)[None, :]
    m = np.zeros((128, 4, CH), np.float32)
    for kd in range(4):
        m[:, kd, :] = np.where(c >= 128 * kd + t, 0.0, -30000.0).astype(np.float32)
    return m


def _fp8_split(x):
    a = x.astype(FP8)
    r = (x - a.astype(np.float32)).astype(FP8)
    return a, r


def prep_core_inputs(inputs, core, s=S, hid=HID):
    b, g = core // 4, core % 4
    heads = slice(HL * g, HL * (g + 1))
    hs = np.asarray(inputs["hidden_states"], np.float32)[b, :s, :hid]
    m = {}
    m["hid8"], m["hidr8"] = _fp8_split(np.ascontiguousarray(hs.T))

    wq = np.asarray(inputs["q_nope_weight"], np.float32).reshape(H, DN, HID)[heads, :, :hid]
    wq_t = wq.transpose(2, 0, 1).reshape(hid, HL * DN)
    wqp = np.asarray(inputs["q_pe_weight"], np.float32).reshape(H, DR, HID)[heads, :, :hid]
    a = wqp[:, _PERM, :]                                   # [4, 64, hid]
    A = a.reshape(2, 128, hid)
    wqpe_t = np.concatenate([A[0], A[1]], axis=0).T
    m["wq8"] = (np.concatenate([wq_t, wqpe_t], axis=1) * WS).astype(FP8)

    wkv = np.asarray(inputs["kv_a_weight"], np.float32)[:, :hid]
    kpe_a = wkv[KVR:][_PERM]
    kpe_b = np.concatenate([-kpe_a[32:], kpe_a[:32]], axis=0)
    wkv_t = np.ascontiguousarray(
        np.concatenate([wkv[:KVR], kpe_a, kpe_b], axis=0).T * WS)
    m["wkv8"], m["wkvr8"] = _fp8_split(wkv_t)

    m["ln_t"] = np.ascontiguousarray(
        np.asarray(inputs["kv_a_ln_weight"], np.float32).reshape(4, 128).T)
    m["kb16"] = (np.asarray(inputs["k_b_weight"], np.float32)[heads] * WS).astype(np.float16)
    m["vb16_t"] = np.ascontiguousarray(
        np.asarray(inputs["v_b_weight"], np.float32)[heads].transpose(0, 2, 1) * WS
    ).astype(np.float16)
    wo_t = np.ascontiguousarray(
        np.asarray(inputs["o_weight"], np.float32)[:hid, HL * DV * g:HL * DV * (g + 1)].T * WS)
    m["wo8_t"], m["wor8_t"] = _fp8_split(wo_t)

    pos = np.asarray(inputs["position_ids"]).reshape(-1)[:s]
    cosT, sinT = _rope_tables(pos, s)                      # [64, s] each
    m["cs2"] = np.ascontiguousarray(np.vstack([cosT, sinT])).astype(BF16)
    m["ckrope"] = np.ascontiguousarray(
        np.vstack([cosT, sinT]) * (1.0 / WS)).astype(BF16)
    m["masks"] = _masks().astype(np.float16)
    m["ident16"] = np.eye(128, dtype=np.float16)
    m["ones_c"] = np.ones((128, 1), np.float32)
    m["ones_r"] = np.ones((1, 128), np.float32)
    m["ones16"] = np.ones((128, 32), np.float16)
    m["ones8"] = np.ones((128, 2, 32), np.float32).astype(FP8)
    return m


_NC_CACHE = {}


def _get_nc():
    if "nc" not in _NC_CACHE:
        _NC_CACHE["nc"] = build_nc()
    return _NC_CACHE["nc"]


def kernel(**inputs):
    from concourse import bass_utils

    nc = _get_nc()
    in_maps = [prep_core_inputs(inputs, c) for c in range(NCORES)]
    res = bass_utils.run_bass_kernel_spmd(nc, in_maps, core_ids=list(range(NCORES)))
    out = np.empty((B, S, HID), np.float32)
    for b in range(B):
        acc = np.array(res.results[4 * b]["out_t"], np.float32)
        for g in range(1, 4):
            acc += res.results[4 * b + g]["out_t"]
        out[b] = acc.T
    return out
